# revision 30
# baseline (speedup 1.0000x reference)
# BLEU loss kernel for Trainium2 (Bass/Tile), 8 NeuronCores, data-parallel over batch.
#
# Reference computation (per sample b, fully independent):
#   gmax[n,k]  = max_g tc_gts[b,g,n,k]
#   judge[n,k] = tc_res[b,0,n,k] <= gmax[n,k]                     (bool output)
#   clip[n]    = sum_k min(tc_res, gmax + 1e-9)
#   l_i = l_res[b,0];  lgf = where(l_gts==0, 200, l_gts)
#   l_r = lgf[argmin_g |l_i - lgf|]   (first min)
#   und = 1/l_i;  r = l_r/l_i  (the +1e-9 vanish in f32 for values >= 1)
#   lfac = exp(1 - max(1, r))
#   gf[n] = clip[n]*und;  ch = {gf, gf+und, relu(gf-und)}
#   flat[81] = outer product of ch over the 4 n-gram orders
#   c_full = lfac * flat^(1/4) = exp(0.25*ln(flat) + (1 - max(1,r)))
#   score = c_full[0]; score_list = c_full - c_full[80]
#   score_dif_l = score * (1*(l_r>l_i) + 0.5*(l_r==l_i)) * l_r * und^2
#
# Transport trick: tc_* values are small ints {0..3} -> host casts to uint8
# (lossless), SWDGE DMA expands u8->bf16 inline. All heavy elementwise ops in
# bf16 (DVE 2x mode); reductions accumulate fp32; the 81-cell grid in fp32.
import numpy as np
from contextlib import ExitStack

B_FULL = 16384
G = 5
N = 4
K = 121
NK = N * K            # 484
GNK = G * NK          # 2420
N_CORES = 8
P = 128
EPS = 1e-9


def _build(b_core, t_chunk):
    import concourse.bass as bass
    import concourse.tile as tile
    import concourse.tile_sem_assignment as tsa
    from concourse import mybir

    # Cap the DMA-completion semaphore lanes so the kernel-tail drain stays
    # within the ISA's embedded sync-wait capacity (one wait per active lane).
    tsa.NUM_SWDGE_GLOBAL_SEMS = 3
    tsa.NUM_HWDGE_SEMS = 2

    f32 = mybir.dt.float32
    bf16 = mybir.dt.bfloat16
    u8 = mybir.dt.uint8
    Alu = mybir.AluOpType
    Act = mybir.ActivationFunctionType
    X = mybir.AxisListType.X

    NT = b_core // P              # tiles (columns) per core
    T = t_chunk
    assert NT % T == 0
    chunk_ts = [(i * T, T) for i in range(NT // T)]
    n_chunks = len(chunk_ts)

    nc = bass.Bass()
    gts_d = nc.declare_dram_parameter("gts", [b_core, GNK], u8, isOutput=False)
    res_d = nc.declare_dram_parameter("res", [b_core, NK], u8, isOutput=False)
    lens_d = nc.declare_dram_parameter("lens", [b_core, G + 1], f32,
                                       isOutput=False)
    judge_d = nc.declare_dram_parameter("judge", [b_core, NK], u8, isOutput=True)
    slist_d = nc.declare_dram_parameter("slist", [b_core, 81], f32, isOutput=True)
    score_d = nc.declare_dram_parameter("score", [b_core], f32, isOutput=True)
    sdif_d = nc.declare_dram_parameter("sdif", [b_core], f32, isOutput=True)

    # sample s = p*NT + t  ->  partition p, column t (keeps every DMA run
    # contiguous per partition: chunk loads read T*row_bytes linearly)
    gts_v = gts_d[:, :].rearrange("(p t) k -> p t k", p=P)      # [128, NT, 2420]
    res_v = res_d[:, :].rearrange("(p t) k -> p t k", p=P)
    lens_v = lens_d[:, :].rearrange("(p t) g -> p t g", p=P)    # [128, NT, 6]
    judge_v = judge_d[:, :].rearrange("(p t) k -> p t k", p=P)
    slist_v = slist_d[:, :].rearrange("(p t) k -> p t k", p=P)
    score_v = score_d[:].rearrange("(p t) -> p t", p=P)
    sdif_v = sdif_d[:].rearrange("(p t) -> p t", p=P)

    with ExitStack() as ctx:
        tc = ctx.enter_context(tile.TileContext(nc))
        lens = ctx.enter_context(tc.tile_pool(name="lens", bufs=1))
        big = ctx.enter_context(tc.tile_pool(name="big", bufs=3))
        pf = ctx.enter_context(tc.tile_pool(name="pf", bufs=3))
        mid = ctx.enter_context(tc.tile_pool(name="mid", bufs=2))
        outp = ctx.enter_context(tc.tile_pool(name="outp", bufs=n_chunks))
        jpool = ctx.enter_context(tc.tile_pool(name="jpool", bufs=n_chunks))
        snt = ctx.enter_context(tc.tile_pool(name="snt", bufs=n_chunks))
        apool = ctx.enter_context(tc.tile_pool(name="apool", bufs=n_chunks))

        # ---------------- whole-core lengths pipeline ----------------
        # l_res and l_gts ride one packed [B,6] tensor: a single small HWDGE
        # DMA on the SP queue lands before the first big SWDGE gts load
        lens_sb = lens.tile([P, NT, G + 1], f32)
        dl = nc.sync.dma_start(out=lens_sb, in_=lens_v)
        lres_sb = lens_sb[:, :, 0]
        lgts_sb = lens_sb[:, :, 1:G + 1]

        recip = lens.tile([P, NT], f32)        # und = 1/l_i
        nc.vector.reciprocal(out=recip[:, :], in_=lres_sb)

        e0 = lens.tile([P, NT, G], f32)
        nc.vector.tensor_scalar(out=e0[:, :, :], in0=lgts_sb,
                                scalar1=0.0, scalar2=None, op0=Alu.is_equal)
        lgf = lens.tile([P, NT, G], f32)
        nc.vector.scalar_tensor_tensor(out=lgf[:, :, :], in0=e0[:, :, :],
                                       scalar=200.0, in1=lgts_sb,
                                       op0=Alu.mult, op1=Alu.add)
        lres_bc = lres_sb.unsqueeze(2).broadcast_to((P, NT, G))
        dist = lens.tile([P, NT, G], f32)
        nc.vector.tensor_tensor(out=dist[:, :, :], in0=lgf[:, :, :],
                                in1=lres_bc, op=Alu.subtract)
        nc.scalar.activation(out=dist[:, :, :], in_=dist[:, :, :], func=Act.Abs)

        best_d = lens.tile([P, NT], f32)
        best_l = lens.tile([P, NT], f32)
        nc.vector.tensor_copy(out=best_d[:, :], in_=dist[:, :, 0])
        nc.vector.tensor_copy(out=best_l[:, :], in_=lgf[:, :, 0])
        mask = lens.tile([P, NT], u8)
        for g in range(1, G):
            nc.vector.tensor_tensor(out=mask[:, :], in0=dist[:, :, g],
                                    in1=best_d[:, :], op=Alu.is_lt)
            nc.vector.copy_predicated(out=best_l[:, :], mask=mask[:, :],
                                      data=lgf[:, :, g])
            nc.vector.tensor_tensor(out=best_d[:, :], in0=best_d[:, :],
                                    in1=dist[:, :, g], op=Alu.min)

        r_ = lens.tile([P, NT], f32)           # r = l_r/l_i (up to 1 ulp)
        nc.vector.tensor_tensor(out=r_[:, :], in0=best_l[:, :],
                                in1=recip[:, :], op=Alu.mult)
        mb = lens.tile([P, NT], f32)           # 1 - max(1, r) = ln(l_factor)
        nc.vector.tensor_scalar(out=mb[:, :], in0=r_[:, :], scalar1=1.0,
                                scalar2=None, op0=Alu.max)
        nc.vector.tensor_scalar(out=mb[:, :], in0=mb[:, :], scalar1=-1.0,
                                scalar2=1.0, op0=Alu.mult, op1=Alu.add)
        lfac = lens.tile([P, NT], f32)
        nc.scalar.activation(out=lfac[:, :], in_=mb[:, :], func=Act.Exp)
        # dif indicator: 1*(l_r>l_i) + 0.5*(l_r==l_i)  (jax max tie-gradient)
        ind_a = lens.tile([P, NT], f32)
        nc.vector.tensor_tensor(out=ind_a[:, :], in0=best_l[:, :],
                                in1=lres_sb, op=Alu.is_gt)
        ind_b = lens.tile([P, NT], f32)
        nc.vector.tensor_tensor(out=ind_b[:, :], in0=best_l[:, :],
                                in1=lres_sb, op=Alu.is_equal)
        ind = lens.tile([P, NT], f32)
        nc.vector.scalar_tensor_tensor(out=ind[:, :], in0=ind_b[:, :],
                                       scalar=0.5, in1=ind_a[:, :],
                                       op0=Alu.mult, op1=Alu.add)
        dif_scale = lens.tile([P, NT], f32)    # ind * l_r * und^2 = ind*r*und
        nc.vector.tensor_tensor(out=dif_scale[:, :], in0=ind[:, :],
                                in1=r_[:, :], op=Alu.mult)
        nc.vector.tensor_tensor(out=dif_scale[:, :], in0=dif_scale[:, :],
                                in1=recip[:, :], op=Alu.mult)

        score_all = lens.tile([P, NT], f32)
        sdif_all = lens.tile([P, NT], f32)

        # keep the scheduler from interleaving the lens ops into the chunk
        # stream: interleaving adds same-engine ordering waits, and combined
        # with a DMA wait that exceeds the ISA's embedded-wait capacity
        tc.no_sync_barrier()

        # ---------------- chunk loop (software-pipelined prefetch) ----------
        # SWDGE DMA order is 4-periodic (gts, res, judge, slist) with 4 sem
        # lanes, so each DMA's lane predecessor is its own kind from the
        # previous chunk. Tiny Pool-engine "observer" copies run right before
        # each output DMA so the DMA itself carries only its lane wait
        # (ISA embedded-wait capacity is 1 for DMA/compute structs).
        from concourse.tile_rust import add_dep_helper

        gts_tiles = {}
        res_tiles = {}
        prev_pss = [None]

        def issue_loads(ci):
            toff, tl = chunk_ts[ci]
            gt = big.tile([P, tl, GNK], bf16, tag="gts_sb")
            d1 = nc.gpsimd.dma_start(out=gt[:, :, :],
                                     in_=gts_v[:, toff:toff + tl, :])
            rt = pf.tile([P, tl, NK], bf16, tag="res_sb")
            d2 = nc.gpsimd.dma_start(out=rt[:, :, :],
                                     in_=res_v[:, toff:toff + tl, :])
            if prev_pss[0] is not None:
                # keep loads behind the previous chunk's Pool observer so
                # their WAR (DVE) waits are already observed -> 1 embedded wait
                add_dep_helper(d1.ins, prev_pss[0].ins, sync=False,
                               reason="load after prev-chunk pool observer")
                add_dep_helper(d2.ins, prev_pss[0].ins, sync=False,
                               reason="load after prev-chunk pool observer")
            gts_tiles[ci] = gt
            res_tiles[ci] = rt

        last_act = [None]
        last_dmas = []

        def stage_a(c):
            """Loads -> maxes -> judge/min -> fold-sum -> grid -> ACT sqrts."""
            toff, T = chunk_ts[c]
            ts = slice(toff, toff + T)
            gts_sb = gts_tiles.pop(c)
            res_sb = res_tiles.pop(c)

            # DVE observers: carry the input-DMA waits alone
            s_g = snt.tile([P, 2], bf16)
            nc.vector.tensor_copy(out=s_g[:, :], in_=gts_sb[:, 0, 0:2])
            s_r = snt.tile([P, 2], bf16)
            nc.vector.tensor_copy(out=s_r[:, :], in_=res_sb[:, 0, 0:2])

            gts4 = gts_sb[:, :, :].rearrange("p t (g k) -> p t g k", g=G)
            m01 = mid.tile([P, T, NK], bf16)
            nc.vector.tensor_tensor(out=m01[:, :, :], in0=gts4[:, :, 0, :],
                                    in1=gts4[:, :, 1, :], op=Alu.max)
            m23 = mid.tile([P, T, NK], bf16)
            nc.vector.tensor_tensor(out=m23[:, :, :], in0=gts4[:, :, 2, :],
                                    in1=gts4[:, :, 3, :], op=Alu.max)
            m03 = mid.tile([P, T, NK], bf16)
            nc.vector.tensor_tensor(out=m03[:, :, :], in0=m01[:, :, :],
                                    in1=m23[:, :, :], op=Alu.max)
            gmax = mid.tile([P, T, NK], bf16)
            nc.vector.tensor_tensor(out=gmax[:, :, :], in0=m03[:, :, :],
                                    in1=gts4[:, :, 4, :], op=Alu.max)

            judge = jpool.tile([P, T, NK], bf16)
            nc.vector.tensor_tensor(out=judge[:, :, :], in0=res_sb[:, :, :],
                                    in1=gmax[:, :, :], op=Alu.is_le)

            minv = mid.tile([P, T, NK], bf16)
            nc.vector.tensor_tensor(out=minv[:, :, :], in0=gmax[:, :, :],
                                    in1=res_sb[:, :, :], op=Alu.min)
            # clip[n] = sum_k minv: binary fold tree (bf16 2x; all partial
            # sums are ints <= 171, exact in bf16), final 3-wide reduce in f32
            minv4 = minv[:, :, :].rearrange("p t (n k) -> p t n k", n=N)
            fA = mid.tile([P, T, N, 60], bf16)
            nc.vector.tensor_tensor(out=fA[:, :, :, :], in0=minv4[:, :, :, 0:60],
                                    in1=minv4[:, :, :, 60:120], op=Alu.add)
            nc.vector.tensor_tensor(out=fA[:, :, :, 0:1], in0=fA[:, :, :, 0:1],
                                    in1=minv4[:, :, :, 120:121], op=Alu.add)
            fB = mid.tile([P, T, N, 30], bf16)
            nc.vector.tensor_tensor(out=fB[:, :, :, :], in0=fA[:, :, :, 0:30],
                                    in1=fA[:, :, :, 30:60], op=Alu.add)
            fC = mid.tile([P, T, N, 15], bf16)
            nc.vector.tensor_tensor(out=fC[:, :, :, :], in0=fB[:, :, :, 0:15],
                                    in1=fB[:, :, :, 15:30], op=Alu.add)
            fD = mid.tile([P, T, N, 7], bf16)
            nc.vector.tensor_tensor(out=fD[:, :, :, :], in0=fC[:, :, :, 0:7],
                                    in1=fC[:, :, :, 7:14], op=Alu.add)
            nc.vector.tensor_tensor(out=fD[:, :, :, 0:1], in0=fD[:, :, :, 0:1],
                                    in1=fC[:, :, :, 14:15], op=Alu.add)
            fE = mid.tile([P, T, N, 3], bf16)
            nc.vector.tensor_tensor(out=fE[:, :, :, :], in0=fD[:, :, :, 0:3],
                                    in1=fD[:, :, :, 3:6], op=Alu.add)
            nc.vector.tensor_tensor(out=fE[:, :, :, 0:1], in0=fE[:, :, :, 0:1],
                                    in1=fD[:, :, :, 6:7], op=Alu.add)
            clip = mid.tile([P, T, N], f32)
            nc.vector.tensor_reduce(out=clip[:, :, :], in_=fE[:, :, :, :],
                                    axis=X, op=Alu.add)

            und_b = recip[:, ts].unsqueeze(2).broadcast_to((P, T, N))
            CH = mid.tile([P, 3, T, N], f32)
            nc.vector.tensor_tensor(out=CH[:, 0], in0=clip[:, :, :],
                                    in1=und_b, op=Alu.mult)
            nc.vector.tensor_tensor(out=CH[:, 1], in0=CH[:, 0],
                                    in1=und_b, op=Alu.add)
            nc.vector.tensor_tensor(out=CH[:, 2], in0=CH[:, 0],
                                    in1=und_b, op=Alu.subtract)
            nc.vector.tensor_scalar(out=CH[:, 2], in0=CH[:, 2], scalar1=0.0,
                                    scalar2=None, op0=Alu.max)

            # e[t,i,j] = CH[i,t,0]*CH[j,t,1];  f[t,k,l] = CH[k,t,2]*CH[l,t,3]
            ef = mid.tile([P, 2, T, 3, 3], f32)
            chv = CH[:, :, :, :]
            for half, (n0, n1) in enumerate(((0, 1), (2, 3))):
                a_i = chv[:, :, :, n0].transpose([0, 2, 1]) \
                    .unsqueeze(3).broadcast_to((P, T, 3, 3))
                b_j = chv[:, :, :, n1].transpose([0, 2, 1]) \
                    .unsqueeze(2).broadcast_to((P, T, 3, 3))
                nc.vector.tensor_tensor(out=ef[:, half], in0=a_i, in1=b_j,
                                        op=Alu.mult)

            flat = mid.tile([P, T, 81], f32)
            e9 = ef[:, 0].rearrange("p t a b -> p t (a b)") \
                .unsqueeze(3).broadcast_to((P, T, 9, 9))
            f9 = ef[:, 1].rearrange("p t a b -> p t (a b)") \
                .unsqueeze(2).broadcast_to((P, T, 9, 9))
            flat4 = flat[:, :, :].rearrange("p t (a b) -> p t a b", a=9)
            nc.vector.tensor_tensor(out=flat4, in0=e9, in1=f9, op=Alu.mult)

            # froot = sqrt(sqrt(flat)) on ACT; consumed one chunk later
            sq1 = apool.tile([P, T, 81], f32)
            nc.scalar.activation(out=sq1[:, :, :], in_=flat[:, :, :],
                                 func=Act.Sqrt)
            sq2 = apool.tile([P, T, 81], f32)
            last_act[0] = nc.scalar.activation(out=sq2[:, :, :],
                                               in_=sq1[:, :, :], func=Act.Sqrt)

            psj = snt.tile([P, 2], bf16)
            oj = nc.gpsimd.tensor_copy(out=psj[:, :], in_=judge[:, 0, 0:2])
            dj = nc.gpsimd.dma_start(out=judge_v[:, ts, :], in_=judge[:, :, :])
            add_dep_helper(dj.ins, oj.ins, sync=False,
                           reason="judge dma after its observer")
            if c == n_chunks - 1:
                last_dmas.append(dj)
            return dict(c=c, ts=ts, sq2=sq2)

        def stage_b(st):
            """cfull/score/score_list/sdif + output DMAs for an earlier chunk."""
            c, ts, sq2 = st["c"], st["ts"], st["sq2"]
            T = ts.stop - ts.start
            # cfull = lfac * froot and sl = cfull - cfull[...,80] run on ACT
            # per tile-column (lfac/c80 become per-partition scale/bias APs)
            cfull = apool.tile([P, T, 81], f32)
            for tt in range(T):
                nc.scalar.activation(out=cfull[:, tt, :], in_=sq2[:, tt, :],
                                     func=Act.Copy,
                                     scale=lfac[:, ts.start + tt:ts.start + tt + 1])
            nc80 = snt.tile([P, T], f32, tag=f"nc80_{c}")
            nc.scalar.activation(out=nc80[:, :], in_=cfull[:, :, 80],
                                 func=Act.Copy, scale=-1.0)
            sl = outp.tile([P, T, 81], f32)
            for tt in range(T):
                last_act[0] = nc.scalar.activation(out=sl[:, tt, :],
                                                   in_=cfull[:, tt, :],
                                                   func=Act.Identity,
                                                   bias=nc80[:, tt:tt + 1])

            nc.vector.tensor_copy(out=score_all[:, ts], in_=cfull[:, :, 0])
            last_dve[0] = nc.vector.tensor_tensor(out=sdif_all[:, ts],
                                                  in0=cfull[:, :, 0],
                                                  in1=dif_scale[:, ts],
                                                  op=Alu.mult)

            # Pool observer + slist output DMA
            pss = snt.tile([P, 2], f32)
            os_ = nc.gpsimd.tensor_copy(out=pss[:, :], in_=sl[:, T - 1, 0:2])
            ds = nc.gpsimd.dma_start(out=slist_v[:, ts, :], in_=sl[:, :, :])
            add_dep_helper(ds.ins, os_.ins, sync=False,
                           reason="slist dma after its observer")
            prev_pss[0] = os_
            if c == n_chunks - 1:
                last_dmas.append(ds)

        last_dve = [None]
        issue_loads(0)
        if n_chunks > 1:
            issue_loads(1)
        pending = None
        for c in range(n_chunks):
            st = stage_a(c)
            if pending is not None:
                stage_b(pending)
            pending = st
            if c + 2 < n_chunks:
                issue_loads(c + 2)
        stage_b(pending)

        psc = snt.tile([P, 2], f32)
        oc = nc.gpsimd.tensor_copy(out=psc[:, :], in_=score_all[:, NT - 2:NT])
        dc = nc.gpsimd.dma_start(out=score_v, in_=score_all[:, :])
        add_dep_helper(dc.ins, oc.ins, sync=False,
                       reason="score dma after its observer")
        psd = snt.tile([P, 2], f32)
        od = nc.gpsimd.tensor_copy(out=psd[:, :], in_=sdif_all[:, NT - 2:NT])
        dd = nc.gpsimd.dma_start(out=sdif_v, in_=sdif_all[:, :])
        add_dep_helper(dd.ins, od.ins, sync=False,
                       reason="sdif dma after its observer")

        # Pre-drain observers: one SP nop per terminal instruction so the
        # kernel-tail drain's per-proc waits are already observed on SP and
        # elide (the drain's embedded-wait capacity is tiny).
        for term in last_dmas + [dc, dd, od, last_act[0], last_dve[0], dl]:
            if term is None:
                continue
            nop = nc.sync.nop()
            add_dep_helper(nop.ins, term.ins, sync=True,
                           reason="pre-drain observer")

    return nc


_CACHE = {}


def _get_program(b_core, t_chunk):
    key = (b_core, t_chunk)
    if key not in _CACHE:
        _CACHE[key] = _build(b_core, t_chunk)
    return _CACHE[key]


def _run(inputs, b_core, t_chunk, n_cores, **run_kwargs):
    from concourse.bass_utils import run_bass_kernel_spmd

    tc_res = np.asarray(inputs["tc_res"])
    tc_gts = np.asarray(inputs["tc_gts"])
    l_res = np.asarray(inputs["l_res"])
    l_gts = np.asarray(inputs["l_gts"])
    b_full = tc_res.shape[0]
    assert b_full == b_core * n_cores

    gts_u8 = np.ascontiguousarray(tc_gts.reshape(b_full, GNK).astype(np.uint8))
    res_u8 = np.ascontiguousarray(tc_res.reshape(b_full, NK).astype(np.uint8))
    lens_h = np.ascontiguousarray(np.concatenate(
        [l_res.reshape(b_full, 1), l_gts.reshape(b_full, G)],
        axis=1).astype(np.float32))

    in_maps = [
        dict(
            gts=gts_u8[i * b_core:(i + 1) * b_core],
            res=res_u8[i * b_core:(i + 1) * b_core],
            lens=lens_h[i * b_core:(i + 1) * b_core],
        )
        for i in range(n_cores)
    ]

    nc = _get_program(b_core, t_chunk)
    kr = run_bass_kernel_spmd(nc, in_maps, core_ids=list(range(n_cores)),
                              **run_kwargs)
    outs = kr.results

    score = np.concatenate([o["score"] for o in outs]).reshape(b_full, 1)
    slist = np.concatenate([o["slist"] for o in outs]) \
        .reshape(b_full, 1, 3, 3, 3, 3)
    judge = np.concatenate([o["judge"] for o in outs]) \
        .reshape(b_full, 1, N, K).astype(bool)
    sdif = np.concatenate([o["sdif"] for o in outs]).reshape(b_full, 1)
    return (score.astype(np.float32), slist.astype(np.float32), judge,
            sdif.astype(np.float32)), kr


def kernel(**inputs):
    outs, _ = _run(inputs, b_core=B_FULL // N_CORES, t_chunk=4,
                   n_cores=N_CORES)
    return outs
